# revision 27
# baseline (speedup 1.0000x reference)
"""AttentionPooling segment-reduce kernel for 8 Trainium2 NeuronCores.

Math (reference):
    k = x @ key_w.T + key_b            # [N, 256] -> heads [N, 4, 64]
    v = x @ value_w.T + value_b
    attn   = einsum('hd,nhd->nh', query, k) * SCALE
    w      = exp(attn)
    wsum   = segment_sum(w)[batch]
    out[b] = segment_sum(w/(wsum+EPS) * v)

Algebraic restructuring (exact):
    attn[n,h] = qt[:,h] . x[n] + sc[h],  qt = SCALE*(key_w^T q per head),
                                         sc = SCALE*(q . key_b per head)
    w = exp(attn) = g[h]*wt[n,h],  wt = exp(qt . x),  g = exp(sc)
    v' = x @ value_w.T                 (bias deferred to segment level)
    St[b,f] = sum_{n in b} wt[n,h(f)] v'[n,f];  dt[b,h] = sum_{n in b} wt[n,h]
    out[b,f] = (St[b,f] + dt[b,h]*value_b[f]) / (dt[b,h] + EPS/g[h])

Device mapping: core c owns segments [c*512,(c+1)*512) split into 4 windows
of 128 segments; window nodes padded to 128-multiples.  Per 128-node tile:
 - PE: fused projection psum[nodes,260] = xT_tile.T @ [Wv^T | qt] (fp16 in,
   fp32 accum), then segment reduce psum_s += onehot.T @ u.  All matmuls
   use the uniform 128x128 tile config (mixed tile_size/col_grp configs
   block the LDWEIGHTS pull-ahead at every transition).
 - ACT: exp of the 4 attn columns per group.
 - PSUM-exit multiply u = v' * wt is split across engines by group:
   route A: one fused DVE tensor_tensor (fp32-PSUM src, 1x mode);
   route B: ACT copies v' to SBUF fp16, GPSIMD does the broadcast multiply
   (tensor_tensor never takes the shared DVE/GpSimd SBUF port pair, so the
   three streams do not contend).
 - Reduce matmuls are emitted LAG groups behind the projection so the PE
   never stalls on the exit pipeline; PSUM group bufs are freed by the
   exit op itself, so the lag only costs SBUF u-tile bufs.
One-hot node->segment matrices are precomputed on the host (exact 0/1 fp16,
64 wide) and streamed alongside x^T in one contiguous HBM chunk per 512
node columns.  Window epilogue (DVE): out = (St + dt*bv) / (dt + eps/g),
DMA to the core's output rows.
"""

from collections import deque
from contextlib import ExitStack

import ml_dtypes
import numpy as np

N = 262144
DIM = 256
H = 4
HD = 64
B = 4096
SCALE = HD ** (-0.5)
EPS = 1e-8

NCORES = 8
SEGS_PER_CORE = B // NCORES          # 512
WSEG = 128                           # segments per window
WPC = SEGS_PER_CORE // WSEG          # 4 windows per core
GRP = 2                              # node-tiles per PSUM group
CHUNK = 1024                         # x columns per DMA chunk (8 tiles)
TPC = CHUNK // 128                   # tiles per chunk
CW = 2 * CHUNK                       # x chunk width in fp16 cols (x0|x1)
OW = TPC * WSEG                      # one-hot chunk width (fp8 cols)
LAG = 6                              # groups the reduce trails by
ROUTE_MOD = 3                        # every ROUTE_MOD-th group -> route B
WARM = 8                             # PE warm-up matmuls (N=512 each)
XBUFS = 10

TRACE = False                        # test harness can flip for profiling
LAST_RESULT = None

_cache = {}


def _build(tws: tuple):
    """Build + compile the SPMD program; tws = node-tiles per window."""
    import concourse.tile as tile
    from concourse import bacc, mybir

    F32 = mybir.dt.float32
    F16 = mybir.dt.float16
    F8 = mybir.dt.float8e4
    Alu = mybir.AluOpType
    Act = mybir.ActivationFunctionType

    TILES = sum(tws)                 # node tiles per core
    NCHUNK = TILES // TPC if TILES % TPC == 0 else TILES // TPC + 1

    nc = bacc.Bacc("TRN2", target_bir_lowering=False, debug=False,
                   num_devices=NCORES)

    pk_d = nc.dram_tensor("pk", [NCHUNK * 128, CW], F16,
                          kind="ExternalInput").ap()
    oh_d = nc.dram_tensor("ohp", [NCHUNK * 128, OW], F8,
                          kind="ExternalInput").ap()
    wq_d = nc.dram_tensor("wq", [128, 520], F16, kind="ExternalInput").ap()
    cst_d = nc.dram_tensor("cst", [128, 260], F32, kind="ExternalInput").ap()
    out_d = nc.dram_tensor("out", [SEGS_PER_CORE, 256], F32,
                           kind="ExternalOutput").ap()

    with tile.TileContext(nc, pool_alloc_mode="queue") as tc, \
            ExitStack() as ctx:
        consts = ctx.enter_context(tc.tile_pool(name="consts", bufs=1))
        xin = ctx.enter_context(tc.tile_pool(name="xin", bufs=XBUFS))
        xoh = ctx.enter_context(tc.tile_pool(name="xoh", bufs=XBUFS))
        up = ctx.enter_context(tc.tile_pool(name="up", bufs=LAG + 3))
        vp = ctx.enter_context(tc.tile_pool(name="vp", bufs=3))
        fxp = ctx.enter_context(tc.tile_pool(name="fxp", bufs=2))
        pp = ctx.enter_context(tc.tile_pool(name="pp", bufs=3, space="PSUM"))
        sp = ctx.enter_context(tc.tile_pool(name="sp", bufs=2, space="PSUM"))

        # PE warm-up: dummy matmuls issued with no dependencies at all
        # (uninitialized SBUF operands; result never read).  They run during
        # the initial input-chunk DMA wait and flip the HAM clock gate to
        # 2.4 GHz before real work arrives.  Kept short: the PE queue is
        # strict FIFO, so excess warm-up delays the first real matmul.
        wtile = consts.tile([128, 512], F16, tag="wtile")
        nc.gpsimd.memset(wtile[:], 0.0)
        wpsum = pp.tile([128, GRP * 512], F32, tag="pp")
        for i in range(WARM):
            bank = (i % 2) * 512
            nc.tensor.matmul(wpsum[:, bank:bank + 512], wtile[:, 0:128],
                             wtile[:], start=True, stop=True)

        wqpk = consts.tile([128, 520], F16, tag="wqpk")
        cst = consts.tile([128, 260], F32, tag="cst")
        nc.sync.dma_start(wqpk[:], wq_d)
        cst_loaded = False
        wq0 = wqpk[:, 0:260]
        wq1 = wqpk[:, 260:520]
        bvrep = cst[:, 0:256]
        epsg = cst[:, 256:260]

        pkt = None
        pend = deque()               # (w, first_t, gsz, ohviews, u4)
        spt = {}                     # window index -> psum_s tile
        gidx = 0
        NG = sum((t + GRP - 1) // GRP for t in tws)

        def emit_reduce(item):
            w, first_t, gsz, ohv, u4 = item
            if w not in spt:
                spt[w] = sp.tile([128, 260], F32, tag="ps", name=f"ps{w}")
            ps = spt[w]
            for b in range(gsz):
                t = first_t + b      # window-local tile index
                nc.tensor.matmul(ps[:], ohv[b],
                                 u4[:, b * 260:(b + 1) * 260],
                                 start=(t == 0), stop=(t == tws[w] - 1))
            if first_t + gsz == tws[w]:
                epilogue(w)

        def epilogue(w):
            ps = spt.pop(w)
            dsum = fxp.tile([128, 4], F32, tag="dsum")
            nc.vector.tensor_tensor(dsum[:], ps[:, 256:260], epsg, Alu.add)
            rec = fxp.tile([128, 4], F32, tag="rec")
            nc.vector.reciprocal(rec[:], dsum[:])
            t1 = fxp.tile([128, 256], F32, tag="t1")
            bv3 = bvrep.rearrange("p (h d) -> p h d", h=H)
            dt3 = (ps[:, 256:260].unsqueeze(2)
                   .broadcast_to([128, H, HD]))
            nc.vector.tensor_tensor(
                t1[:].rearrange("p (h d) -> p h d", h=H), bv3, dt3, Alu.mult)
            t2 = fxp.tile([128, 256], F32, tag="t2")
            nc.vector.tensor_tensor(t2[:], ps[:, 0:256], t1[:], Alu.add)
            outt = fxp.tile([128, 256], F32, tag="outt")
            rec3 = rec[:].unsqueeze(2).broadcast_to([128, H, HD])
            nc.vector.tensor_tensor(
                outt[:].rearrange("p (h d) -> p h d", h=H),
                t2[:].rearrange("p (h d) -> p h d", h=H), rec3, Alu.mult)
            nc.sync.dma_start(out_d[w * 128:(w + 1) * 128, :], outt[:])

        tcore = 0                    # core-global tile counter
        for w in range(WPC):
            tw = tws[w]
            for gi, g0 in enumerate(range(0, tw, GRP)):
                gsz = min(GRP, tw - g0)
                psum4 = pp.tile([128, gsz * 512], F32, tag="pp")
                u4 = up.tile([128, gsz * 260], F16, tag="u4")
                ohview = []
                for b in range(gsz):
                    if tcore % TPC == 0:
                        ci = tcore // TPC
                        pkt = xin.tile([128, CW], F16, tag="pkt")
                        oht = xoh.tile([128, OW], F8, tag="oht")
                        if ci == 0:
                            s0 = CHUNK + 128
                            nc.sync.dma_start(pkt[:, 0:s0],
                                              pk_d[0:128, 0:s0])
                            nc.sync.dma_start(pkt[:, s0:CW],
                                              pk_d[0:128, s0:CW])
                        else:
                            nc.sync.dma_start(
                                pkt[:], pk_d[ci * 128:(ci + 1) * 128, :])
                        nc.sync.dma_start(
                            oht[:], oh_d[ci * 128:(ci + 1) * 128, :])
                    if not cst_loaded and gidx >= 4:
                        cst_loaded = True
                        nc.sync.dma_start(cst[:], cst_d)
                    o = (tcore % TPC) * 128
                    ps = psum4[:, b * 512:b * 512 + 260]
                    nc.tensor.matmul(ps, pkt[:, o:o + 128], wq0,
                                     start=True, stop=False)
                    nc.tensor.matmul(ps, pkt[:, CHUNK + o:CHUNK + o + 128],
                                     wq1, start=False, stop=True)
                    ohview.append(
                        oht[:, (tcore % TPC) * WSEG:
                            (tcore % TPC) * WSEG + WSEG])
                    tcore += 1

                p3 = psum4[:].rearrange("p (b c) -> p b c", c=512)
                u3 = u4[:].rearrange("p (b c) -> p b c", c=260)
                nc.scalar.activation(u3[:, :, 256:260], p3[:, :, 256:260],
                                     Act.Exp)
                in1 = (u3[:, :, 256:260].unsqueeze(3)
                       .broadcast_to([128, gsz, H, HD]))
                o4 = u3[:, :, 0:256].rearrange("p b (h d) -> p b h d", h=H)
                tailpos = NG - gidx
                if tailpos <= 6:
                    route_b = (tailpos % 2 == 0)
                else:
                    route_b = (gidx % ROUTE_MOD == ROUTE_MOD - 1)
                if route_b:
                    vsb = vp.tile([128, gsz * 256], F16, tag="vsb")
                    v3 = vsb[:].rearrange("p (b c) -> p b c", c=256)
                    nc.scalar.copy(v3, p3[:, :, 0:256])
                    iv = v3.rearrange("p b (h d) -> p b h d", h=H)
                    nc.gpsimd.tensor_tensor(o4, iv, in1, Alu.mult)
                else:
                    in0 = (p3[:, :, 0:256]
                           .rearrange("p b (h d) -> p b h d", h=H))
                    nc.vector.tensor_tensor(o4, in0, in1, Alu.mult)
                gidx += 1

                pend.append((w, g0, gsz, ohview, u4))
                lag = LAG if gidx < NG - 8 else 1
                while len(pend) > lag:
                    emit_reduce(pend.popleft())
        while pend:
            emit_reduce(pend.popleft())

    nc.compile()
    return nc


def kernel(x, batch, query, key_w, key_b, value_w, value_b):
    global LAST_RESULT
    from concourse.bass_utils import run_bass_kernel_spmd

    x = np.asarray(x, dtype=np.float32)
    batch = np.asarray(batch).astype(np.int64)
    query = np.asarray(query, dtype=np.float32)
    key_w = np.asarray(key_w, dtype=np.float32)
    key_b = np.asarray(key_b, dtype=np.float32)
    value_w = np.asarray(value_w, dtype=np.float32)
    value_b = np.asarray(value_b, dtype=np.float32)

    # ---- host-side planning ----
    counts = np.bincount(batch, minlength=B)
    cum = np.zeros(B + 1, np.int64)
    cum[1:] = np.cumsum(counts)
    nwin = NCORES * WPC
    wstart = cum[np.arange(nwin) * WSEG]
    wend = cum[(np.arange(nwin) + 1) * WSEG]
    tiles_w = (wend - wstart + 127) // 128
    # per-window-index tile count = max across cores (one shared program)
    tws = tuple(int(tiles_w.reshape(NCORES, WPC)[:, w].max())
                for w in range(WPC))
    woff = np.concatenate([[0], np.cumsum(tws)])   # window tile offsets
    TILES = int(sum(tws))
    P = TILES * 128
    NCHUNK = (TILES + TPC - 1) // TPC

    # ---- shared constants ----
    wqf = np.zeros((256, 260), np.float32)
    wqf[:, 0:256] = value_w.T
    qt = (key_w.reshape(H, HD, DIM) * query[:, :, None]).sum(axis=1)  # [H,256]
    wqf[:, 256:260] = SCALE * qt.T
    wq = np.concatenate([wqf[0:128], wqf[128:256]],
                        axis=1).astype(np.float16)          # [128, 520]
    sc = SCALE * (query * key_b.reshape(H, HD)).sum(axis=1)           # [H]
    g = np.exp(sc).astype(np.float32)
    cst = np.zeros((128, 260), np.float32)
    cst[:, 0:256] = value_b
    cst[:, 256:260] = EPS / g

    # ---- per-core shards ----
    PC = NCHUNK * TPC * 128          # chunk-padded node columns
    in_maps = []
    for c in range(NCORES):
        xTp = np.zeros((256, PC), np.float16)
        ohp = np.zeros((128, NCHUNK * TPC, WSEG), np.float16)
        for w in range(WPC):
            m = c * WPC + w
            ns, ne = int(wstart[m]), int(wend[m])
            L = ne - ns
            col0 = int(woff[w]) * 128
            xTp[:, col0:col0 + L] = x[ns:ne, :].T.astype(np.float16)
            j = (batch[ns:ne] - m * WSEG).astype(np.int64)
            node = np.arange(L) + col0
            ohp[node % 128, node // 128, j] = np.float16(1.0)
        pk = np.zeros((NCHUNK, 128, CW), np.float16)
        xc = xTp.reshape(256, NCHUNK, TPC * 128)
        pk[:, :, 0:CHUNK] = xc[0:128].transpose(1, 0, 2)
        pk[:, :, CHUNK:2 * CHUNK] = xc[128:256].transpose(1, 0, 2)
        oh8 = (ohp.reshape(128, NCHUNK, OW).transpose(1, 0, 2)
               .astype(ml_dtypes.float8_e4m3))
        in_maps.append({"pk": pk.reshape(NCHUNK * 128, CW),
                        "ohp": oh8.reshape(NCHUNK * 128, OW),
                        "wq": wq, "cst": cst})

    if tws not in _cache:
        _cache[tws] = _build(tws)
    nc = _cache[tws]

    res = run_bass_kernel_spmd(nc, in_maps, core_ids=list(range(NCORES)),
                               trace=TRACE)
    LAST_RESULT = res
    return np.concatenate([r["out"] for r in res.results], axis=0)
